# revision 1
# baseline (speedup 1.0000x reference)
"""GNN message-passing kernel for Trainium2 (8 NeuronCores, batch-parallel).

Computation (per reference):
    norm_adj = adjacency * dinv * dinv.T + I            [10,10]   (host, O(100) flops)
    support  = einsum('bcf,fo->bco', x, kernel)         [B,C,O]
    out      = elu(einsum('ij,bjo->bio', norm_adj, support) + bias)
    out      = (out - mean) * rsqrt(var+eps) * gamma + beta

Device strategy per core (512 batches = 5120 rows of [b,c] x f):
  1. "Transposing mix": PE matmul with x-chunks [crows<=120, 128f] as the
     stationary operand and a block-diagonal norm_adj matrix [crows, 256pad]
     as the moving operand. One op both applies the channel mix and lands
     the activations transposed ([f, rows]) as needed by the main matmul.
  2. Main matmul, fp32r full-rate: outT[o,rows] += K[f,o].T @ yT[f,rows],
     kernel matrix resident in SBUF.
  3. Epilogue on ACT/DVE with per-partition (o) params:
     elu(z) = min(exp(z), relu(z)+1) - 1 (exact), then folded BN affine.
     Output stored transposed [O, rows]; host transposes while unsharding.
"""

from contextlib import ExitStack

import numpy as np

import concourse.bass as bass
import concourse.bacc as bacc
import concourse.mybir as mybir
import concourse.tile as tile
from concourse.bass_utils import run_bass_kernel_spmd

F32 = mybir.dt.float32
F32R = mybir.dt.float32r
ALU = mybir.AluOpType
ACTF = mybir.ActivationFunctionType

P = 128
BN_EPS = 1e-3
N_CORES = 8
C = 10  # channels
BDW = 256  # mix moving-operand pad width


def build_nc(rows, F, O, chunk_batches=(12, 12, 8), n_cores=N_CORES, repeats=1):
    """Build the per-core Bass program. rows = local (b,c) rows, F/O = feat dims.

    repeats>1 replays the whole computation (for timing amplification)."""
    panel = sum(chunk_batches) * C  # rows per panel (multiple of 10, >=256)
    assert rows % panel == 0
    n_panels = rows // panel
    FT, OT = F // P, O // P
    bd_sizes = sorted({nb * C for nb in chunk_batches})

    nc = bacc.Bacc(
        "TRN2",
        target_bir_lowering=False,
        debug=False,
        enable_asserts=False,
        num_devices=n_cores,
    )
    x_d = nc.dram_tensor("x_local", [rows, F], F32, kind="ExternalInput").ap()
    k_d = nc.dram_tensor("kern", [F, O], F32, kind="ExternalInput").ap()
    # blob packs the small constants into one DMA: per bd size 256 cols, then
    # prm cols [0:OT]=bias_t, [OT:2OT]=scale_t, [2OT:3OT]=shift2_t (per-partition o)
    blob_cols = BDW * len(bd_sizes) + 3 * OT
    blob_d = nc.dram_tensor("blob", [P, blob_cols], F32, kind="ExternalInput").ap()
    outT_d = nc.dram_tensor("outT", [O, rows], F32, kind="ExternalOutput").ap()

    with tile.TileContext(nc) as tc, ExitStack() as ctx:
        const = ctx.enter_context(tc.tile_pool(name="const", bufs=1))
        blob = const.tile([P, blob_cols], F32R, name="blob")
        nc.sync.dma_start(blob, blob_d.bitcast(F32R))
        bd_t = {
            sz: blob[:sz, BDW * i : BDW * (i + 1)]
            for i, sz in enumerate(bd_sizes)
        }
        prm = blob[:, BDW * len(bd_sizes) :].bitcast(F32)
        kb = [const.tile([P, O], F32R, name=f"kb{fb}", tag=f"kb{fb}") for fb in range(FT)]
        for fb in range(FT):
            nc.scalar.dma_start(kb[fb], k_d[fb * P : (fb + 1) * P, :].bitcast(F32R))

        xpool = ctx.enter_context(tc.tile_pool(name="xpool", bufs=3))
        ypool = ctx.enter_context(tc.tile_pool(name="ypool", bufs=1))
        mixps = ctx.enter_context(tc.tile_pool(name="mixps", bufs=2, space="PSUM"))
        mainps = ctx.enter_context(tc.tile_pool(name="mainps", bufs=4, space="PSUM"))
        tmp = ctx.enter_context(tc.tile_pool(name="tmp", bufs=2))

        for rep in range(repeats):
          for pi in range(n_panels):
            row0 = pi * panel
            ytall = ypool.tile([P, FT, panel], F32R, name=f"r{rep}_yt_{pi}", tag="yt")
            # ---- mix phase: yT[f, rows_panel] = blockdiag(normadj) applied to x
            coff = 0
            for ci, nb in enumerate(chunk_batches):
                crows = nb * C
                xt = xpool.tile([120, F], F32R, name=f"r{rep}_x_{pi}_{ci}", tag="xc")[:crows]
                nc.sync.dma_start(xt, x_d[row0 + coff : row0 + coff + crows, :].bitcast(F32R))
                for fbp in range(FT // 4):
                    fb = 4 * fbp
                    ps = mixps.tile([P, 4, BDW], F32, name=f"r{rep}_mps_{pi}_{ci}_{fbp}", tag="mixps")
                    for q in range(4):
                        nc.tensor.matmul(
                            ps[:, q, :],
                            lhsT=xt[:, (fb + q) * P : (fb + q + 1) * P],
                            rhs=bd_t[crows],
                            start=True,
                            stop=True,
                        )
                    nc.vector.tensor_copy(
                        ytall[:, fb : fb + 4, coff : coff + crows], ps[:, :, :crows]
                    )
                coff += crows
            # ---- main matmul + epilogue per o-tile
            for ot in range(OT):
                ps = mainps.tile([P, panel], F32, name=f"r{rep}_ops_{pi}_{ot}", tag="mainps")
                for fb in range(FT):
                    nc.tensor.matmul(
                        ps,
                        lhsT=kb[fb][:, ot * P : (ot + 1) * P],
                        rhs=ytall[:, fb, :],
                        start=(fb == 0),
                        stop=(fb == FT - 1),
                    )
                bias_ap = prm[:, ot : ot + 1]
                scale_ap = prm[:, OT + ot : OT + ot + 1]
                shift_ap = prm[:, 2 * OT + ot : 2 * OT + ot + 1]
                e = tmp.tile([P, panel], F32, name=f"r{rep}_e_{pi}_{ot}", tag="e")
                t0 = tmp.tile([P, panel], F32, name=f"r{rep}_t0_{pi}_{ot}", tag="t0")
                s = tmp.tile([P, panel], F32, name=f"r{rep}_s_{pi}_{ot}", tag="s")
                fin = tmp.tile([P, panel], F32, name=f"r{rep}_fin_{pi}_{ot}", tag="fin")
                nc.scalar.activation(e, ps, ACTF.Exp, bias=bias_ap)
                nc.scalar.activation(t0, ps, ACTF.Relu, bias=bias_ap)
                # elu(zb) + 1 = min(exp(zb), relu(zb) + 1)   (exact identity)
                nc.vector.scalar_tensor_tensor(
                    s, in0=t0, scalar=1.0, in1=e, op0=ALU.add, op1=ALU.min
                )
                # fin = s*scale + (shift - scale) = elu*scale + shift
                nc.vector.tensor_scalar(
                    fin, s, scale_ap, shift_ap, op0=ALU.mult, op1=ALU.add
                )
                nc.scalar.dma_start(outT_d[ot * P : (ot + 1) * P, row0 : row0 + panel], fin)
    nc.compile()
    return nc


def _host_prep(adjacency, kern, bias, gamma, beta, moving_mean, moving_var,
               chunk_batches=(12, 12, 8), O=2048):
    """Build the tiny derived inputs on the host."""
    A = np.asarray(adjacency, np.float32)
    deg = np.maximum(np.abs(A).sum(axis=1, keepdims=True), 1e-8)
    dinv = deg ** -0.5
    na = A * dinv * dinv.T + np.eye(C, dtype=np.float32)  # [10,10]

    bd_sizes = sorted({nb * C for nb in chunk_batches})
    OT = O // P
    blob = np.zeros((P, BDW * len(bd_sizes) + 3 * OT), np.float32)
    for i, sz in enumerate(bd_sizes):
        nb = sz // C
        for g in range(nb):
            blob[g * C : (g + 1) * C, BDW * i + g * C : BDW * i + (g + 1) * C] = na.T
    scale = np.asarray(gamma, np.float32) / np.sqrt(np.asarray(moving_var, np.float32) + BN_EPS)
    shift2 = np.asarray(beta, np.float32) - np.asarray(moving_mean, np.float32) * scale - scale
    p0 = BDW * len(bd_sizes)
    blob[:, p0 : p0 + OT] = np.asarray(bias, np.float32).reshape(OT, P).T
    blob[:, p0 + OT : p0 + 2 * OT] = scale.reshape(OT, P).T
    blob[:, p0 + 2 * OT : p0 + 3 * OT] = shift2.reshape(OT, P).T
    return blob


def kernel(x, adjacency, kernel, bias, gamma, beta, moving_mean, moving_var):
    B, C_, F = x.shape
    O = kernel.shape[1]
    assert C_ == C
    assert B % N_CORES == 0
    bl = B // N_CORES
    rows = bl * C

    chunk_batches = (12, 12, 8)
    blob = _host_prep(adjacency, kernel, bias, gamma, beta, moving_mean,
                      moving_var, chunk_batches, O)

    nc = build_nc(rows, F, O, chunk_batches)

    kern_np = np.ascontiguousarray(np.asarray(kernel, np.float32))
    x_np = np.asarray(x, np.float32)
    in_maps = []
    for c in range(N_CORES):
        in_maps.append({
            "x_local": np.ascontiguousarray(x_np[c * bl : (c + 1) * bl].reshape(rows, F)),
            "kern": kern_np,
            "blob": blob,
        })

    res = run_bass_kernel_spmd(nc, in_maps, core_ids=list(range(N_CORES)), trace=False)

    out = np.empty((B, C, O), np.float32)
    for c in range(N_CORES):
        outT = res.results[c]["outT"]  # [O, rows]
        out[c * bl : (c + 1) * bl] = outT.T.reshape(bl, C, O)
    return out



# revision 5
# speedup vs baseline: 1.1914x; 1.1914x over previous
"""GNN message-passing kernel for Trainium2 (8 NeuronCores, batch-parallel).

Computation (per reference):
    norm_adj = adjacency * dinv * dinv.T + I            [10,10]   (host, O(100) flops)
    support  = einsum('bcf,fo->bco', x, kernel)         [B,C,O]
    out      = elu(einsum('ij,bjo->bio', norm_adj, support) + bias)
    out      = (out - mean) * rsqrt(var+eps) * gamma + beta

Device strategy per core (512 batches = 5120 rows of [b,c] x f), bf16 operands
(rel-err ~3e-3, well under the 2e-2 gate; bf16 runs the PE at full rate, halves
DMA traffic, and enables fast weight loads):
  1. "Transposing mix": PE matmul with x-chunks [crows<=120, 128f] as the
     stationary operand and a block-diagonal norm_adj [crows, crows] as the
     moving operand. One op applies the channel mix and lands the activations
     transposed ([f, rows]) as needed by the main matmul.
  2. Main matmul: outT[o,rows] += K[f,o].T @ yT[f,rows], kernel resident in
     SBUF (bf16). Mix for panel p+1 is emitted before main for panel p so the
     PE never waits on the PSUM->SBUF copies of the mix results.
  3. Epilogue on ACT/DVE with per-partition (o) params:
     elu(z) = min(exp(z), relu(z)+1) - 1 (exact), then folded BN affine.
     Output stored transposed [O, rows] bf16; host transposes/casts while
     unsharding. Output DMA rides the otherwise-idle Pool queue so the ACT
     queue only carries the activation ops.
"""

from contextlib import ExitStack

import numpy as np
import ml_dtypes

import concourse.bass as bass
import concourse.bacc as bacc
import concourse.mybir as mybir
import concourse.tile as tile
from concourse.bass_utils import run_bass_kernel_spmd

F32 = mybir.dt.float32
BF16 = mybir.dt.bfloat16
ALU = mybir.AluOpType
ACTF = mybir.ActivationFunctionType

P = 128
BN_EPS = 1e-3
N_CORES = 8
C = 10  # channels
CHUNKS = (12, 12, 8)  # batches per mix chunk (x10 rows each), sums to panel/C


def build_nc(rows, F, O, chunk_batches=CHUNKS, n_cores=N_CORES, repeats=1, warmup=4):
    """Build the per-core Bass program. rows = local (b,c) rows, F/O = feat dims.

    warmup = how many panels of mix run ahead of the main matmul (covers the
    startup window while the 8MB kernel matrix streams in)."""
    panel = sum(chunk_batches) * C  # rows per panel
    assert rows % panel == 0
    n_panels = rows // panel
    FT, OT = F // P, O // P
    bd_sizes = sorted({nb * C for nb in chunk_batches})
    bd_off = {}
    off = 0
    for sz in bd_sizes:
        bd_off[sz] = off
        off += sz
    mixb_cols = off

    nc = bacc.Bacc(
        "TRN2",
        target_bir_lowering=False,
        debug=False,
        enable_asserts=False,
        num_devices=n_cores,
    )
    x_d = nc.dram_tensor("x_local", [rows, F], BF16, kind="ExternalInput").ap()
    k_d = nc.dram_tensor("kern", [F, O], BF16, kind="ExternalInput").ap()
    mixb_d = nc.dram_tensor("mixb", [P, mixb_cols], BF16, kind="ExternalInput").ap()
    # prm cols [0:OT]=bias_t, [OT:2OT]=scale_t, [2OT:3OT]=shift2_t (per-partition o)
    prm_d = nc.dram_tensor("prm", [P, 3 * OT], F32, kind="ExternalInput").ap()
    outT_d = nc.dram_tensor("outT", [O, rows], BF16, kind="ExternalOutput").ap()

    with tile.TileContext(nc) as tc, ExitStack() as ctx:
        const = ctx.enter_context(tc.tile_pool(name="const", bufs=1))
        mixb = const.tile([P, mixb_cols], BF16, name="mixb")
        prm = const.tile([P, 3 * OT], F32, name="prm")
        nc.sync.dma_start(mixb, mixb_d)
        nc.sync.dma_start(prm, prm_d)
        bd_t = {sz: mixb[:sz, bd_off[sz] : bd_off[sz] + sz] for sz in bd_sizes}
        kb = [const.tile([P, O], BF16, name=f"kb{fb}", tag=f"kb{fb}") for fb in range(FT)]
        for fb in range(FT):
            nc.gpsimd.dma_start(kb[fb], k_d[fb * P : (fb + 1) * P, :])

        xpool = ctx.enter_context(tc.tile_pool(name="xpool", bufs=6))
        ypool = ctx.enter_context(tc.tile_pool(name="ypool", bufs=warmup + 2))
        mixps = ctx.enter_context(tc.tile_pool(name="mixps", bufs=2, space="PSUM"))
        mainps = ctx.enter_context(tc.tile_pool(name="mainps", bufs=4, space="PSUM"))
        tmp = ctx.enter_context(tc.tile_pool(name="tmp", bufs=2))

        yts = {}

        def emit_mix(rep, pi):
            row0 = pi * panel
            ytall = ypool.tile([P, FT, panel], BF16, name=f"r{rep}_yt_{pi}", tag="yt")
            yts[(rep, pi)] = ytall
            coff = 0
            for ci, nb in enumerate(chunk_batches):
                crows = nb * C
                xt = xpool.tile([120, F], BF16, name=f"r{rep}_x_{pi}_{ci}", tag="xc")[:crows]
                nc.sync.dma_start(xt, x_d[row0 + coff : row0 + coff + crows, :])
                for fbp in range(FT // 4):
                    fb = 4 * fbp
                    ps = mixps.tile([P, 4, 120], F32, name=f"r{rep}_mps_{pi}_{ci}_{fbp}", tag="mixps")
                    for q in range(4):
                        nc.tensor.matmul(
                            ps[:, q, :crows],
                            lhsT=xt[:, (fb + q) * P : (fb + q + 1) * P],
                            rhs=bd_t[crows],
                            start=True,
                            stop=True,
                        )
                    nc.vector.tensor_copy(
                        ytall[:, fb : fb + 4, coff : coff + crows], ps[:, :, :crows]
                    )
                coff += crows

        def emit_main(rep, pi):
            row0 = pi * panel
            ytall = yts.pop((rep, pi))
            for ot in range(OT):
                ps = mainps.tile([P, panel], F32, name=f"r{rep}_ops_{pi}_{ot}", tag="mainps")
                for fb in range(FT):
                    nc.tensor.matmul(
                        ps,
                        lhsT=kb[fb][:, ot * P : (ot + 1) * P],
                        rhs=ytall[:, fb, :],
                        start=(fb == 0),
                        stop=(fb == FT - 1),
                    )
                bias_ap = prm[:, ot : ot + 1]
                scale_ap = prm[:, OT + ot : OT + ot + 1]
                shift_ap = prm[:, 2 * OT + ot : 2 * OT + ot + 1]
                e = tmp.tile([P, panel], F32, name=f"r{rep}_e_{pi}_{ot}", tag="e")
                t0 = tmp.tile([P, panel], F32, name=f"r{rep}_t0_{pi}_{ot}", tag="t0")
                s = tmp.tile([P, panel], F32, name=f"r{rep}_s_{pi}_{ot}", tag="s")
                fin = tmp.tile([P, panel], BF16, name=f"r{rep}_fin_{pi}_{ot}", tag="fin")
                nc.scalar.activation(e, ps, ACTF.Exp, bias=bias_ap)
                nc.scalar.activation(t0, ps, ACTF.Relu, bias=bias_ap)
                # elu(zb) + 1 = min(exp(zb), relu(zb) + 1)   (exact identity)
                nc.vector.scalar_tensor_tensor(
                    s, in0=t0, scalar=1.0, in1=e, op0=ALU.add, op1=ALU.min
                )
                # fin = s*scale + (shift - scale) = elu*scale + shift
                nc.vector.tensor_scalar(
                    fin, s, scale_ap, shift_ap, op0=ALU.mult, op1=ALU.add
                )
                nc.gpsimd.dma_start(outT_d[ot * P : (ot + 1) * P, row0 : row0 + panel], fin)

        for rep in range(repeats):
            w = warmup if rep == 0 else 1
            for pi in range(n_panels + w):
                if pi < n_panels:
                    emit_mix(rep, pi)
                if pi >= w:
                    emit_main(rep, pi - w)
    nc.compile()
    return nc


def _host_prep(adjacency, kern, bias, gamma, beta, moving_mean, moving_var,
               chunk_batches=CHUNKS, O=2048):
    """Build the tiny derived inputs on the host: mixb (bf16) and prm (f32)."""
    A = np.asarray(adjacency, np.float32)
    deg = np.maximum(np.abs(A).sum(axis=1, keepdims=True), 1e-8)
    dinv = deg ** -0.5
    na = A * dinv * dinv.T + np.eye(C, dtype=np.float32)  # [10,10]

    bd_sizes = sorted({nb * C for nb in chunk_batches})
    OT = O // P
    mixb_cols = sum(bd_sizes)
    mixb = np.zeros((P, mixb_cols), np.float32)
    off = 0
    for sz in bd_sizes:
        nb = sz // C
        for g in range(nb):
            mixb[g * C : (g + 1) * C, off + g * C : off + (g + 1) * C] = na.T
        off += sz
    scale = np.asarray(gamma, np.float32) / np.sqrt(np.asarray(moving_var, np.float32) + BN_EPS)
    shift2 = np.asarray(beta, np.float32) - np.asarray(moving_mean, np.float32) * scale - scale
    prm = np.empty((P, 3 * OT), np.float32)
    prm[:, :OT] = np.asarray(bias, np.float32).reshape(OT, P).T
    prm[:, OT : 2 * OT] = scale.reshape(OT, P).T
    prm[:, 2 * OT :] = shift2.reshape(OT, P).T
    return mixb.astype(ml_dtypes.bfloat16), prm


def make_in_maps(x, adjacency, kernel, bias, gamma, beta, moving_mean, moving_var):
    B, C_, F = x.shape
    O = kernel.shape[1]
    bl = B // N_CORES
    rows = bl * C
    mixb, prm = _host_prep(adjacency, kernel, bias, gamma, beta, moving_mean,
                           moving_var, CHUNKS, O)
    kern_np = np.ascontiguousarray(np.asarray(kernel, np.float32).astype(ml_dtypes.bfloat16))
    x_bf = np.asarray(x, np.float32).astype(ml_dtypes.bfloat16)
    in_maps = []
    for c in range(N_CORES):
        in_maps.append({
            "x_local": np.ascontiguousarray(x_bf[c * bl : (c + 1) * bl].reshape(rows, F)),
            "kern": kern_np,
            "mixb": mixb,
            "prm": prm,
        })
    return in_maps


def unshard(outT_per_core, B, O):
    """outT_per_core: list/array of [O, rows] bf16 per core -> [B, C, O] f32."""
    bl = B // N_CORES
    rows = bl * C
    out = np.empty((B, C, O), np.float32)
    for c in range(N_CORES):
        outT = np.asarray(outT_per_core[c], dtype=np.float32)  # [O, rows]
        out[c * bl : (c + 1) * bl] = outT.T.reshape(bl, C, O)
    return out


def kernel(x, adjacency, kernel, bias, gamma, beta, moving_mean, moving_var):
    B, C_, F = x.shape
    O = kernel.shape[1]
    assert C_ == C
    assert B % N_CORES == 0
    bl = B // N_CORES
    rows = bl * C

    in_maps = make_in_maps(x, adjacency, kernel, bias, gamma, beta,
                           moving_mean, moving_var)
    nc = build_nc(rows, F, O, CHUNKS)
    res = run_bass_kernel_spmd(nc, in_maps, core_ids=list(range(N_CORES)), trace=False)
    return unshard([res.results[c]["outT"] for c in range(N_CORES)], B, O)
